# revision 6
# baseline (speedup 1.0000x reference)
"""Trainium2 Bass kernel: DepthSeparableConv2d block.

reference semantics:
    y = relu(bn1(depthwise3x3(x) + dw_b));  y = prune(y, 4.0)   per (b,c)
    z = relu(bn2(pointwise1x1(y) + pw_b));  z = prune(z, 0.001) per (b,o)

Strategy (8 NeuronCores, data-parallel over batch; channel = partition):
  - BN affines folded into conv weights/biases on the host (float64).
  - x lives in SBUF as [128, 58 rows x 56 cols] - H-padded only, so the
    input DMA lands as one contiguous 12.5KB/partition block (full DMA
    line rate).  Horizontal taps run on width-55 column windows instead
    of a W-pad (the pad column contribution is zero by definition).
  - Depthwise 3x3:
      * the three center-column taps (kx=1) on TensorE as fp32
        diag-weight matmuls accumulating in PSUM per 448-wide tile,
      * the six side-column taps on VectorE (one 2x tensor_scalar init +
        five in-place scalar_tensor_tensor fp32 MACs),
      * a custom DVE op merges PSUM + SBUF accumulators, adds the bias,
        applies ReLU, and max-reduces per partition in ONE 1x pass
        (prune1 comes out for free).
  - prune1 mask folded into the pointwise lhsT (zeroed rows).
  - pointwise matmul in float32r (1 cyc/row vs fp32's 4; HW-measured
    ~2.5e-4 relative on this kernel, well inside tolerance).
  - BN2+relu fused into one ScalarE activation per PSUM tile with
    accum_out per-tile sums; prune2 via sum>=thr (== max>=thr except for
    all-tiny channels; error bounded by thr=1e-3 and never over-prunes).
  - prune2 mask applied by ScalarE (activation Copy with per-partition
    scale) to keep VectorE free.
"""

import os
import sys

import numpy as np

sys.path.insert(0, "/opt/trn_rl_repo")

import concourse.bacc as bacc  # noqa: E402
import concourse.tile as tile  # noqa: E402
from concourse import mybir  # noqa: E402
from concourse.bass_utils import run_bass_kernel_spmd  # noqa: E402


def _install_ntff_hook():
    """Register the axon NTFF profile hook (the image's antenv lacks
    axon_hooks, so trace=True would otherwise silently skip profiling)."""
    import types

    if "antenv.axon_hooks" in sys.modules:
        return
    mod = types.ModuleType("antenv.axon_hooks")
    state = {"hook": None}
    mod.set_axon_ntff_profile_hook = lambda h: state.__setitem__("hook", h)
    mod.get_axon_ntff_profile_hook = lambda: state["hook"]
    sys.modules["antenv.axon_hooks"] = mod
    try:
        if "/root/.axon_site" not in sys.path:
            sys.path.append("/root/.axon_site")
        from trn_agent_boot.trn_boot import _ntff_profile_via_ctypes

        hook = _ntff_profile_via_ctypes("/opt/axon/libaxon_pjrt.so")
        mod.set_axon_ntff_profile_hook(hook)
    except Exception:
        pass


_install_ntff_hook()

EPS = 1e-5
DW_THR = 4.0
PW_THR = 0.001

N_CORES = 8
B, C, O, H, W = 64, 128, 256, 56, 56
BL = B // N_CORES  # batches per core
HR = H + 2  # padded row count (58)
S = H * W  # 3136
TSP = 448  # spatial tile (8 rows of 56)
NT = S // TSP  # 7

PE_TAPS = (1, 4, 7)  # center-column taps (kx=1) on TensorE
DVE_STT_TAPS = (2, 3, 5, 6, 8)  # remaining side taps (tap 0 is the TS init)

_CACHE: dict = {}


def _register_fused_op():
    """Custom DVE op: out = relu(in0*s0 + in1 + s1);
    accum_out = max(0, max(out)).

    Depthwise merge: in0 = PSUM partial (PE taps), s0 = 1.0, in1 = SBUF
    partial (DVE taps), s1 = folded BN1 bias.  One 1x VectorE pass
    replaces {PSUM merge, bias add, ScalarE relu pass, VectorE
    reduce_max} and feeds prune1.
    """
    from concourse import dve_ops as dvo
    from concourse.dve_spec import (
        C0,
        C1,
        Spec,
        Src0,
        Src1,
        Zero,
        lower,
        maxx,
        relu,
    )
    from concourse.dve_uop import DveOpSpec

    name = "AFFINE_ADD_RELU_MAXACC_ANT"
    if name in dvo._SUB_OPCODE_FOR_NAME:
        return next(op for op in dvo.OPS if op.name == name)

    def ref(in0, in1, s0, s1, imm2):
        out = np.maximum(in0.astype(np.float32) * s0 + in1 + s1, 0.0)
        acc = np.maximum(
            out.reshape(out.shape[0], -1).max(axis=-1, keepdims=True), 0.0
        )
        return out, acc

    spec = Spec(
        body=relu(Src0 * C0 + Src1 + C1),
        accum=maxx,
        accum_init=Zero,
        reference=ref,
    )
    row = dvo._CUSTOM_DVE_ROW_BASE + len(dvo.OPS)
    shas = {
        ver: DveOpSpec(
            name=name, opcode=row, uops=lower(spec, ver=ver), rd1_en=True
        ).sha(ver)
        for ver in ("v3", "v4")
    }
    op = dvo.DveOp(name, spec, subdim=False, uops_sha=shas)
    dvo.OPS.append(op)
    dvo.CUSTOM_DVE_SPECS[name] = spec
    dvo._SUB_OPCODE_FOR_NAME[name] = row
    return op


def _tap_views(xf, yv, k):
    """x window and y (out/in1) window for tap k on the H-pad-only layout.

    kx=0 reads x[.., w-1] -> valid for out cols 1..55 (col 0 gets zero
    from the virtual pad); kx=2 reads x[.., w+1] -> out cols 0..54.
    """
    ky, kx = divmod(k, 3)
    if kx == 0:
        return xf[:, ky : ky + H, 0 : W - 1], yv[:, :, 1:W]
    if kx == 2:
        return xf[:, ky : ky + H, 1:W], yv[:, :, 0 : W - 1]
    return xf[:, ky : ky + H, :], yv[:, :, :]


def build_nc():
    f32 = mybir.dt.float32
    f32r = mybir.dt.float32r
    AX = mybir.AxisListType
    AL = mybir.AluOpType
    AF = mybir.ActivationFunctionType
    fused_op = _register_fused_op()

    nc = bacc.Bacc(
        "TRN2",
        target_bir_lowering=False,
        debug=False,
        num_devices=N_CORES,
    )

    x_d = nc.dram_tensor("x", [BL, C, H, W], f32, kind="ExternalInput").ap()
    par_d = nc.dram_tensor("par", [C, 16], f32, kind="ExternalInput").ap()
    pw_d = nc.dram_tensor("pw", [C, O], f32, kind="ExternalInput").ap()
    dg_d = nc.dram_tensor(
        "dg", [C, len(PE_TAPS) * C], f32, kind="ExternalInput"
    ).ap()
    z_d = nc.dram_tensor("z", [BL, O, H, W], f32, kind="ExternalOutput").ap()

    with tile.TileContext(nc) as tc:
        with (
            tc.tile_pool(name="const", bufs=1) as cpool,
            tc.tile_pool(name="xp", bufs=2) as xpool,
            tc.tile_pool(name="y", bufs=2) as ypool,
            tc.tile_pool(name="yr", bufs=2) as yrpool,
            tc.tile_pool(name="zh", bufs=3) as zpool,
            tc.tile_pool(name="wb", bufs=2) as wbpool,
            tc.tile_pool(name="sm", bufs=32) as smpool,
            tc.tile_pool(name="pdw", bufs=3, space="PSUM") as pdwpool,
            tc.tile_pool(name="ppw", bufs=4, space="PSUM") as ppwpool,
        ):
            par = cpool.tile([C, 16], f32, tag="par")
            nc.sync.dma_start(par[:], par_d)
            pw = cpool.tile([C, O], f32, tag="pw")
            nc.sync.dma_start(pw[:], pw_d)
            dg = cpool.tile([C, len(PE_TAPS) * C], f32, tag="dg")
            nc.sync.dma_start(dg[:], dg_d)

            for b in range(BL):
                xp = xpool.tile([C, HR * W], f32, tag="xp")
                xf = xp[:].rearrange("p (h w) -> p h w", h=HR)
                nc.gpsimd.memset(xf[:, 0:1, :], 0.0)
                nc.gpsimd.memset(xf[:, HR - 1 : HR, :], 0.0)
                # contiguous 12.5KB/partition load into rows 1..56
                nc.sync.dma_start(xf[:, 1 : H + 1, :], x_d[b])

                # depthwise: VectorE accumulator (side-column taps)
                y = ypool.tile([C, S], f32, tag="y")
                yv = y[:].rearrange("p (h w) -> p h w", h=H)
                # col 0 is untouched by the kx=0 init tap; zero it first
                nc.vector.memset(yv[:, :, 0:1], 0.0)
                xin, yout = _tap_views(xf, yv, 0)
                nc.vector.tensor_scalar(yout, xin, par[:, 0:1], None, AL.mult)
                for k in DVE_STT_TAPS:
                    xin, yout = _tap_views(xf, yv, k)
                    nc.vector.scalar_tensor_tensor(
                        yout, xin, par[:, k : k + 1], yout, AL.mult, AL.add
                    )

                # depthwise: TensorE center taps into PSUM per spatial tile,
                # then the fused DVE op merges + bias + relu + per-tile max.
                yr = yrpool.tile([C, S], f32r, tag="yr")
                m1s = smpool.tile([C, NT], f32, tag="m1s")
                for j in range(NT):
                    pdw = pdwpool.tile([C, TSP], f32, tag="pdw")
                    for t, k in enumerate(PE_TAPS):
                        ky = k // 3
                        nc.tensor.matmul(
                            pdw[:],
                            lhsT=dg[:, t * C : (t + 1) * C],
                            rhs=xf[:, 8 * j + ky : 8 * j + ky + 8, :],
                            start=(t == 0),
                            stop=(t == len(PE_TAPS) - 1),
                        )
                    nc.vector._custom_dve(
                        fused_op,
                        out=yr[:, j * TSP : (j + 1) * TSP],
                        in0=pdw[:],
                        in1=y[:, j * TSP : (j + 1) * TSP],
                        s0=1.0,
                        s1=par[:, 9:10],
                        accum_out=m1s[:, j : j + 1],
                    )

                # prune1 mask -> masked pointwise weights (float32r)
                m1 = smpool.tile([C, 1], f32, tag="m1")
                nc.vector.tensor_reduce(m1[:], m1s[:], AX.X, AL.max)
                k1 = smpool.tile([C, 1], f32, tag="k1")
                nc.vector.tensor_scalar(k1[:], m1[:], DW_THR, None, AL.is_ge)
                wb = wbpool.tile([C, O], f32r, tag="wb")
                nc.vector.tensor_scalar(wb[:], pw[:], k1[:], None, AL.mult)

                for o2 in range(2):
                    zh = zpool.tile([C, S], f32, tag="zh")
                    zs = smpool.tile([C, NT], f32, tag="zs")
                    for j in range(NT):
                        ppw = ppwpool.tile([C, TSP], f32, tag="ppw")
                        nc.tensor.matmul(
                            ppw[:],
                            lhsT=wb[:, o2 * C : (o2 + 1) * C],
                            rhs=yr[:, j * TSP : (j + 1) * TSP],
                            start=True,
                            stop=True,
                        )
                        nc.scalar.activation(
                            zh[:, j * TSP : (j + 1) * TSP],
                            ppw[:],
                            AF.Relu,
                            bias=par[:, 10 + o2 : 11 + o2],
                            scale=1.0,
                            accum_out=zs[:, j : j + 1],
                        )
                    zt = smpool.tile([C, 1], f32, tag="zt")
                    nc.vector.tensor_reduce(zt[:], zs[:], AX.X, AL.add)
                    k2 = smpool.tile([C, 1], f32, tag="k2")
                    nc.vector.tensor_scalar(k2[:], zt[:], PW_THR, None, AL.is_ge)
                    # prune2 applied on ScalarE (Copy w/ per-partition scale)
                    nc.scalar.mul(zh[:], zh[:], k2[:])
                    nc.sync.dma_start(
                        z_d[b, o2 * C : (o2 + 1) * C],
                        zh[:].rearrange("p (h w) -> p h w", h=H),
                    )

    nc.compile()
    return nc


def fold_params(inp: dict):
    """Fold BN affines into conv weights/biases (float64 folds)."""
    f8 = np.float64
    dw_w = np.asarray(inp["dw_w"], f8)  # [C,1,3,3]
    dw_b = np.asarray(inp["dw_b"], f8)
    g1, b1, m1, v1 = (np.asarray(inp[k], f8) for k in ("g1", "b1", "m1", "v1"))
    pw_w = np.asarray(inp["pw_w"], f8)  # [O,C,1,1]
    pw_b = np.asarray(inp["pw_b"], f8)
    g2, b2, m2, v2 = (np.asarray(inp[k], f8) for k in ("g2", "b2", "m2", "v2"))

    inv1 = g1 / np.sqrt(v1 + EPS)  # [C]
    wtap = dw_w[:, 0].reshape(C, 9) * inv1[:, None]  # [C,9]
    b1p = dw_b * inv1 + (b1 - m1 * inv1)  # [C]

    inv2 = g2 / np.sqrt(v2 + EPS)  # [O]
    lhsT = (pw_w[:, :, 0, 0] * inv2[:, None]).T  # [C,O]
    b2p = pw_b * inv2 + (b2 - m2 * inv2)  # [O]

    par = np.zeros((C, 16), np.float32)
    par[:, 0:9] = wtap.astype(np.float32)
    par[:, 9] = b1p.astype(np.float32)
    par[:, 10] = b2p[:C].astype(np.float32)
    par[:, 11] = b2p[C:].astype(np.float32)

    dgm = np.zeros((C, len(PE_TAPS) * C), np.float32)
    for t, k in enumerate(PE_TAPS):
        dgm[np.arange(C), t * C + np.arange(C)] = wtap[:, k].astype(np.float32)
    return par, lhsT.astype(np.float32), dgm


def kernel(**inputs) -> np.ndarray:
    x = np.ascontiguousarray(np.asarray(inputs["x"], np.float32))
    assert x.shape == (B, C, H, W)
    par, pw, dgm = fold_params(inputs)

    if "nc" not in _CACHE:
        _CACHE["nc"] = build_nc()
    nc = _CACHE["nc"]

    in_maps = [
        {"x": x[i * BL : (i + 1) * BL], "par": par, "pw": pw, "dg": dgm}
        for i in range(N_CORES)
    ]
    trace = bool(int(os.environ.get("KERNEL_TRACE", "0")))
    res = run_bass_kernel_spmd(nc, in_maps, list(range(N_CORES)), trace=trace)
    _CACHE["last_exec_time_ns"] = res.exec_time_ns

    z = np.empty((B, O, H, W), np.float32)
    for i in range(N_CORES):
        z[i * BL : (i + 1) * BL] = res.results[i]["z"]
    return z


# revision 10
# speedup vs baseline: 1.0835x; 1.0835x over previous
"""Trainium2 Bass kernel: DepthSeparableConv2d block.

reference semantics:
    y = relu(bn1(depthwise3x3(x) + dw_b));  y = prune(y, 4.0)   per (b,c)
    z = relu(bn2(pointwise1x1(y) + pw_b));  z = prune(z, 0.001) per (b,o)

Strategy (8 NeuronCores, data-parallel over batch; channel = partition):
  - BN affines folded into conv weights/biases on the host (float64).
  - x lives in SBUF as [128, 58 rows x 56 cols] - H-padded only, so the
    input DMA lands as one contiguous 12.5KB/partition block (full DMA
    line rate).  Horizontal taps run on width-55 column windows instead
    of a W-pad (the pad column contribution is zero by definition).
  - Depthwise 3x3:
      * the three center-column taps (kx=1) on TensorE as fp32
        diag-weight matmuls accumulating in PSUM per 448-wide tile,
      * the six side-column taps on VectorE (one 2x tensor_scalar init +
        five in-place scalar_tensor_tensor fp32 MACs),
      * a custom DVE op merges PSUM + SBUF accumulators, adds the bias,
        applies ReLU, and max-reduces per partition in ONE 1x pass
        (prune1 comes out for free).
  - prune1 mask folded into the pointwise lhsT (zeroed rows).
  - pointwise matmul in float32r (1 cyc/row vs fp32's 4; HW-measured
    ~2.5e-4 relative on this kernel, well inside tolerance).
  - BN2+relu fused into one ScalarE activation per PSUM tile with
    accum_out per-tile sums; prune2 via sum>=thr (== max>=thr except for
    all-tiny channels; error bounded by thr=1e-3 and never over-prunes).
  - prune2 mask applied by ScalarE (activation Copy with per-partition
    scale) to keep VectorE free.
"""

import os
import sys

import numpy as np

sys.path.insert(0, "/opt/trn_rl_repo")

import concourse.bacc as bacc  # noqa: E402
import concourse.tile as tile  # noqa: E402
from concourse import mybir  # noqa: E402
from concourse.bass_utils import run_bass_kernel_spmd  # noqa: E402


def _install_ntff_hook():
    """Register the axon NTFF profile hook (the image's antenv lacks
    axon_hooks, so trace=True would otherwise silently skip profiling)."""
    import types

    if "antenv.axon_hooks" in sys.modules:
        return
    mod = types.ModuleType("antenv.axon_hooks")
    state = {"hook": None}
    mod.set_axon_ntff_profile_hook = lambda h: state.__setitem__("hook", h)
    mod.get_axon_ntff_profile_hook = lambda: state["hook"]
    sys.modules["antenv.axon_hooks"] = mod
    try:
        if "/root/.axon_site" not in sys.path:
            sys.path.append("/root/.axon_site")
        from trn_agent_boot.trn_boot import _ntff_profile_via_ctypes

        hook = _ntff_profile_via_ctypes("/opt/axon/libaxon_pjrt.so")
        mod.set_axon_ntff_profile_hook(hook)
    except Exception:
        pass


_install_ntff_hook()

EPS = 1e-5
DW_THR = 4.0
PW_THR = 0.001

N_CORES = 8
B, C, O, H, W = 64, 128, 256, 56, 56
BL = B // N_CORES  # batches per core
HR = H + 2  # padded row count (58)
S = H * W  # 3136
TSP = 448  # spatial tile (8 rows of 56)
NT = S // TSP  # 7

PE_TAPS = (1, 4, 7)  # center-column taps (kx=1) on TensorE
DVE_STT_TAPS = (2, 3, 5, 6, 8)  # remaining side taps (tap 0 is the TS init)

_CACHE: dict = {}


def _register_fused_op():
    """Custom DVE op: out = relu(in0*s0 + in1 + s1);
    accum_out = max(0, max(out)).

    Depthwise merge: in0 = PSUM partial (PE taps), s0 = 1.0, in1 = SBUF
    partial (DVE taps), s1 = folded BN1 bias.  One 1x VectorE pass
    replaces {PSUM merge, bias add, ScalarE relu pass, VectorE
    reduce_max} and feeds prune1.
    """
    from concourse import dve_ops as dvo
    from concourse.dve_spec import (
        C0,
        C1,
        Spec,
        Src0,
        Src1,
        Zero,
        lower,
        maxx,
        relu,
    )
    from concourse.dve_uop import DveOpSpec

    name = "AFFINE_ADD_RELU_MAXACC_ANT"
    if name in dvo._SUB_OPCODE_FOR_NAME:
        return next(op for op in dvo.OPS if op.name == name)

    def ref(in0, in1, s0, s1, imm2):
        out = np.maximum(in0.astype(np.float32) * s0 + in1 + s1, 0.0)
        acc = np.maximum(
            out.reshape(out.shape[0], -1).max(axis=-1, keepdims=True), 0.0
        )
        return out, acc

    spec = Spec(
        body=relu(Src0 * C0 + Src1 + C1),
        accum=maxx,
        accum_init=Zero,
        reference=ref,
    )
    row = dvo._CUSTOM_DVE_ROW_BASE + len(dvo.OPS)
    shas = {
        ver: DveOpSpec(
            name=name, opcode=row, uops=lower(spec, ver=ver), rd1_en=True
        ).sha(ver)
        for ver in ("v3", "v4")
    }
    op = dvo.DveOp(name, spec, subdim=False, uops_sha=shas)
    dvo.OPS.append(op)
    dvo.CUSTOM_DVE_SPECS[name] = spec
    dvo._SUB_OPCODE_FOR_NAME[name] = row
    return op


def _tap_views(xf, yv, k):
    """x window and y (out/in1) window for tap k on the H-pad-only layout.

    kx=0 reads x[.., w-1] -> valid for out cols 1..55 (col 0 gets zero
    from the virtual pad); kx=2 reads x[.., w+1] -> out cols 0..54.
    """
    ky, kx = divmod(k, 3)
    if kx == 0:
        return xf[:, ky : ky + H, 0 : W - 1], yv[:, :, 1:W]
    if kx == 2:
        return xf[:, ky : ky + H, 1:W], yv[:, :, 0 : W - 1]
    return xf[:, ky : ky + H, :], yv[:, :, :]


def build_nc():
    f32 = mybir.dt.float32
    f32r = mybir.dt.float32r
    AX = mybir.AxisListType
    AL = mybir.AluOpType
    AF = mybir.ActivationFunctionType
    fused_op = _register_fused_op()

    nc = bacc.Bacc(
        "TRN2",
        target_bir_lowering=False,
        debug=False,
        num_devices=N_CORES,
    )

    x_d = nc.dram_tensor("x", [BL, C, H, W], f32, kind="ExternalInput").ap()
    par_d = nc.dram_tensor("par", [C, 16], f32, kind="ExternalInput").ap()
    pw_d = nc.dram_tensor("pw", [C, O], f32, kind="ExternalInput").ap()
    dg_d = nc.dram_tensor(
        "dg", [C, len(PE_TAPS) * C], f32, kind="ExternalInput"
    ).ap()
    z_d = nc.dram_tensor("z", [BL, O, H, W], f32, kind="ExternalOutput").ap()

    with tile.TileContext(nc) as tc:
        with (
            tc.tile_pool(name="const", bufs=1) as cpool,
            tc.tile_pool(name="xp", bufs=3) as xpool,
            tc.tile_pool(name="y", bufs=3) as ypool,
            tc.tile_pool(name="yr", bufs=3) as yrpool,
            tc.tile_pool(name="zh", bufs=3) as zpool,
            tc.tile_pool(name="wb", bufs=2) as wbpool,
            tc.tile_pool(name="sm", bufs=32) as smpool,
            tc.tile_pool(name="pdw", bufs=4, space="PSUM") as pdwpool,
            tc.tile_pool(name="ppw", bufs=2, space="PSUM") as ppwpool,
        ):
            par = cpool.tile([C, 16], f32, tag="par")
            nc.sync.dma_start(par[:], par_d)
            pw = cpool.tile([C, O], f32, tag="pw")
            nc.sync.dma_start(pw[:], pw_d)
            dg = cpool.tile([C, len(PE_TAPS) * C], f32, tag="dg")
            nc.sync.dma_start(dg[:], dg_d)

            for b in range(BL):
                xp = xpool.tile([C, HR * W], f32, tag="xp")
                xf = xp[:].rearrange("p (h w) -> p h w", h=HR)
                nc.gpsimd.memset(xf[:, 0:1, :], 0.0)
                nc.gpsimd.memset(xf[:, HR - 1 : HR, :], 0.0)
                # contiguous 12.5KB/partition load into rows 1..56
                nc.sync.dma_start(xf[:, 1 : H + 1, :], x_d[b])

                # depthwise: VectorE accumulator (side-column taps)
                y = ypool.tile([C, S], f32, tag="y")
                yv = y[:].rearrange("p (h w) -> p h w", h=H)
                # col 0 is untouched by the kx=0 init tap; zero it first
                nc.vector.memset(yv[:, :, 0:1], 0.0)
                xin, yout = _tap_views(xf, yv, 0)
                nc.vector.tensor_scalar(yout, xin, par[:, 0:1], None, AL.mult)
                for k in DVE_STT_TAPS:
                    xin, yout = _tap_views(xf, yv, k)
                    nc.vector.scalar_tensor_tensor(
                        yout, xin, par[:, k : k + 1], yout, AL.mult, AL.add
                    )

                # depthwise: TensorE center taps into PSUM per spatial tile,
                # then the fused DVE op merges + bias + relu + per-tile max.
                yr = yrpool.tile([C, S], f32r, tag="yr")
                m1s = smpool.tile([C, NT], f32, tag="m1s")
                for j in range(NT):
                    pdw = pdwpool.tile([C, TSP], f32, tag="pdw")
                    for t, k in enumerate(PE_TAPS):
                        ky = k // 3
                        nc.tensor.matmul(
                            pdw[:],
                            lhsT=dg[:, t * C : (t + 1) * C],
                            rhs=xf[:, 8 * j + ky : 8 * j + ky + 8, :],
                            start=(t == 0),
                            stop=(t == len(PE_TAPS) - 1),
                        )
                    nc.vector._custom_dve(
                        fused_op,
                        out=yr[:, j * TSP : (j + 1) * TSP],
                        in0=pdw[:],
                        in1=y[:, j * TSP : (j + 1) * TSP],
                        s0=1.0,
                        s1=par[:, 9:10],
                        accum_out=m1s[:, j : j + 1],
                    )

                # prune1 mask -> masked pointwise weights (float32r)
                m1 = smpool.tile([C, 1], f32, tag="m1")
                nc.vector.tensor_reduce(m1[:], m1s[:], AX.X, AL.max)
                k1 = smpool.tile([C, 1], f32, tag="k1")
                nc.vector.tensor_scalar(k1[:], m1[:], DW_THR, None, AL.is_ge)
                wb = wbpool.tile([C, O], f32r, tag="wb")
                nc.vector.tensor_scalar(wb[:], pw[:], k1[:], None, AL.mult)

                # pointwise: PSUM tiles paired (2 banks) so one ScalarE
                # activation covers 896 elements (halves the per-op +
                # accumulator-readout overhead)
                groups = [(0, 1), (2, 3), (4, 5), (6,)]
                for o2 in range(2):
                    zh = zpool.tile([C, S], f32, tag="zh")
                    zs = smpool.tile([C, len(groups)], f32, tag="zs")
                    for gi, grp in enumerate(groups):
                        # one 448-wide matmul per 512-elem PSUM bank
                        ppw = ppwpool.tile([C, 1024], f32, tag="ppw")
                        pv = ppw[:].rearrange("p (g t) -> p g t", g=2)
                        for gj, j in enumerate(grp):
                            nc.tensor.matmul(
                                pv[:, gj : gj + 1, 0:TSP],
                                lhsT=wb[:, o2 * C : (o2 + 1) * C],
                                rhs=yr[:, j * TSP : (j + 1) * TSP],
                                start=True,
                                stop=True,
                            )
                        width = len(grp) * TSP
                        dst = zh[
                            :, grp[0] * TSP : grp[0] * TSP + width
                        ].rearrange("p (g t) -> p g t", t=TSP)
                        nc.scalar.activation(
                            dst,
                            pv[:, 0 : len(grp), 0:TSP],
                            AF.Relu,
                            bias=par[:, 10 + o2 : 11 + o2],
                            scale=1.0,
                            accum_out=zs[:, gi : gi + 1],
                        )
                    zt = smpool.tile([C, 1], f32, tag="zt")
                    nc.vector.tensor_reduce(zt[:], zs[:], AX.X, AL.add)
                    k2 = smpool.tile([C, 1], f32, tag="k2")
                    nc.vector.tensor_scalar(k2[:], zt[:], PW_THR, None, AL.is_ge)
                    # prune2 applied on ScalarE (Copy w/ per-partition scale)
                    nc.scalar.mul(zh[:], zh[:], k2[:])
                    nc.sync.dma_start(
                        z_d[b, o2 * C : (o2 + 1) * C],
                        zh[:].rearrange("p (h w) -> p h w", h=H),
                    )

    nc.compile()
    return nc


def fold_params(inp: dict):
    """Fold BN affines into conv weights/biases (float64 folds)."""
    f8 = np.float64
    dw_w = np.asarray(inp["dw_w"], f8)  # [C,1,3,3]
    dw_b = np.asarray(inp["dw_b"], f8)
    g1, b1, m1, v1 = (np.asarray(inp[k], f8) for k in ("g1", "b1", "m1", "v1"))
    pw_w = np.asarray(inp["pw_w"], f8)  # [O,C,1,1]
    pw_b = np.asarray(inp["pw_b"], f8)
    g2, b2, m2, v2 = (np.asarray(inp[k], f8) for k in ("g2", "b2", "m2", "v2"))

    inv1 = g1 / np.sqrt(v1 + EPS)  # [C]
    wtap = dw_w[:, 0].reshape(C, 9) * inv1[:, None]  # [C,9]
    b1p = dw_b * inv1 + (b1 - m1 * inv1)  # [C]

    inv2 = g2 / np.sqrt(v2 + EPS)  # [O]
    lhsT = (pw_w[:, :, 0, 0] * inv2[:, None]).T  # [C,O]
    b2p = pw_b * inv2 + (b2 - m2 * inv2)  # [O]

    par = np.zeros((C, 16), np.float32)
    par[:, 0:9] = wtap.astype(np.float32)
    par[:, 9] = b1p.astype(np.float32)
    par[:, 10] = b2p[:C].astype(np.float32)
    par[:, 11] = b2p[C:].astype(np.float32)

    dgm = np.zeros((C, len(PE_TAPS) * C), np.float32)
    for t, k in enumerate(PE_TAPS):
        dgm[np.arange(C), t * C + np.arange(C)] = wtap[:, k].astype(np.float32)
    return par, lhsT.astype(np.float32), dgm


def kernel(**inputs) -> np.ndarray:
    x = np.ascontiguousarray(np.asarray(inputs["x"], np.float32))
    assert x.shape == (B, C, H, W)
    par, pw, dgm = fold_params(inputs)

    if "nc" not in _CACHE:
        _CACHE["nc"] = build_nc()
    nc = _CACHE["nc"]

    in_maps = [
        {"x": x[i * BL : (i + 1) * BL], "par": par, "pw": pw, "dg": dgm}
        for i in range(N_CORES)
    ]
    trace = bool(int(os.environ.get("KERNEL_TRACE", "0")))
    res = run_bass_kernel_spmd(nc, in_maps, list(range(N_CORES)), trace=trace)
    _CACHE["last_exec_time_ns"] = res.exec_time_ns

    z = np.empty((B, O, H, W), np.float32)
    for i in range(N_CORES):
        z[i * BL : (i + 1) * BL] = res.results[i]["z"]
    return z
